# revision 13
# baseline (speedup 1.0000x reference)
"""BiMamba block (fwd + bwd Mamba on [2, 1024, 1024]) for 8 Trainium2 NeuronCores.

Sharding: core = (batch b, direction d, channel-half h)  ->  c = b*4 + d*2 + h.
Each core runs one full Mamba direction on one batch element with half the
d_inner channels (1024 of 2048).  The only cross-core exchange is a 2-core
AllReduce of the x-projection partials ([96, L] fp32) between the two
channel-halves of the same (batch, direction).  The depthwise conv, SSM scan,
and gating are channel-local.  Final out-proj partials ([D_MODEL, L] fp32 per
core) are summed on the host.

Layout on chip: channels on partitions (8 j-tiles of 128), sequence L on the
free dim.  The selective scan runs as DVE tensor_tensor_scan ops over glued
[128, NB*(L+1)] tiles (n-states side by side, zero seam column resets the
recurrence between states).
"""

import numpy as np

# ---------------------------------------------------------------- config ----

FULL = dict(DM=1024, DI=2048, L=1024, NN=16, R=64, KC=4)

N_CORES = 8
NB = 4          # n-states per glued scan group
F16 = "float16"  # on-chip low-precision dtype


# ------------------------------------------------------------- program ------

def build_program(DM, DI, L, NN, R, KC, use_silu=True, n_cores=N_CORES,
                  no_collective=False):
    """Emit the per-core Tile program (SPMD, identical on all cores)."""
    import concourse.bass as bass
    import concourse.mybir as mybir
    import concourse.tile as tile
    from concourse import bacc

    dt = mybir.dt
    f32 = dt.float32
    f16 = getattr(dt, F16)
    AF = mybir.ActivationFunctionType
    OP = mybir.AluOpType

    DL = DI // 2          # local d_inner channels
    NJ = DL // 128        # channel tiles
    KJ = DM // 128        # d_model tiles
    PROJ = R + 2 * NN     # 96
    W1 = L + 1            # glued per-state width (incl. seam)
    NGRP = NN // NB
    NH = max(L // 512, 1) # 512-wide matmul halves
    NW = min(L, 512)

    nc = bacc.Bacc("TRN2", target_bir_lowering=False, debug=False,
                   num_devices=n_cores)

    dram = lambda name, shape, d, kind: nc.dram_tensor(name, shape, d, kind=kind).ap()
    xT_d = dram("xT", [DM, L], f16, "ExternalInput")
    inwT_d = dram("inwT", [DM, 2 * DL], f16, "ExternalInput")
    xprojT_d = dram("xprojT", [DL, PROJ], f16, "ExternalInput")
    dtwT_d = dram("dtwT", [R, DL], f16, "ExternalInput")
    outwT_d = dram("outwT", [DL, DM], f16, "ExternalInput")
    ddiag_d = dram("ddiag", [NJ, 128, 128], f16, "ExternalInput")
    ident_d = dram("ident", [128, 128], f16, "ExternalInput")
    # packed per-j params: cols 0:NN A | NN:NN+KC convw | +convb | +dtb | +Dp
    PPRM = NN + KC + 3
    prm_d = dram("prm", [NJ, 128, PPRM], f32, "ExternalInput")
    out_d = dram("out", [DM, L], f32, "ExternalOutput")

    with tile.TileContext(nc) as tc:
        import contextlib
        ctx = contextlib.ExitStack()
        with ctx:
            # ---------------- persistent pools ----------------
            pers = ctx.enter_context(tc.tile_pool(name="pers", bufs=1))
            dramp = ctx.enter_context(tc.tile_pool(name="dram", bufs=1, space="DRAM"))

            xc = [pers.tile([128, L], f16, name=f"xc{j}", tag=f"xc{j}") for j in range(NJ)]
            zs = [pers.tile([128, L], f16, name=f"zs{j}", tag=f"zs{j}") for j in range(NJ)]
            delta = [pers.tile([128, L], f16, name=f"dl{j}", tag=f"dl{j}") for j in range(NJ)]
            du = [pers.tile([128, L], f16, name=f"du{j}", tag=f"du{j}") for j in range(NJ)]
            prm = [pers.tile([128, PPRM], f32, name=f"pr{j}", tag=f"pr{j}") for j in range(NJ)]
            ident = pers.tile([128, 128], f16, name="ident", tag="ident")
            nc.sync.dma_start(ident[:], ident_d[:])
            one_t = pers.tile([128, 1], f32, name="one", tag="one")
            nc.vector.memset(one_t[:], 1.0)
            for j in range(NJ):
                nc.sync.dma_start(prm[j][:], prm_d[j])
            A_ap = lambda j, n: prm[j][:, n:n + 1]
            convw_ap = lambda j, k: prm[j][:, NN + k:NN + k + 1]
            convb_ap = lambda j: prm[j][:, NN + KC:NN + KC + 1]
            dtb_ap = lambda j: prm[j][:, NN + KC + 1:NN + KC + 2]
            Dp_ap = lambda j: prm[j][:, NN + KC + 2:NN + KC + 3]

            projf = pers.tile([PROJ, L], f32, name="projf", tag="projf")
            projh = pers.tile([PROJ, L], f16, name="projh", tag="projh")
            rows_dram = dramp.tile([2 * NN, L], f16)

            # ---------------- stage A: in_proj + conv + silu ----------------
            psP = ctx.enter_context(tc.tile_pool(name="psP", bufs=1, space="PSUM"))
            ps_proj = psP.tile([PROJ, L], f32)

            with tc.tile_pool(name="xk", bufs=1) as xkp, \
                 tc.tile_pool(name="wk", bufs=1) as wkp, \
                 tc.tile_pool(name="xpw", bufs=1) as xpwp, \
                 tc.tile_pool(name="psA", bufs=3, space="PSUM") as psA, \
                 tc.tile_pool(name="cnv", bufs=2) as cnv:
                xk = []
                for kt in range(KJ):
                    t = xkp.tile([128, L], f16, name=f"xk{kt}", tag=f"xk{kt}")
                    nc.sync.dma_start(t[:], xT_d[kt * 128:(kt + 1) * 128, :])
                    xk.append(t)
                wk = []
                for kt in range(KJ):
                    t = wkp.tile([128, 2 * DL], f16, name=f"wk{kt}", tag=f"wk{kt}")
                    nc.sync.dma_start(t[:], inwT_d[kt * 128:(kt + 1) * 128, :])
                    wk.append(t)
                xpw = []
                for j in range(NJ):
                    t = xpwp.tile([128, PROJ], f16, name=f"xpw{j}", tag=f"xpw{j}")
                    nc.sync.dma_start(t[:], xprojT_d[j * 128:(j + 1) * 128, :])
                    xpw.append(t)

                for mt in range(2 * NJ):
                    ps = psA.tile([128, L], f32, name="psA", tag="psA")
                    for kt in range(KJ):
                        for hh in range(NH):
                            nc.tensor.matmul(
                                ps[:, hh * NW:(hh + 1) * NW],
                                wk[kt][:, mt * 128:(mt + 1) * 128],
                                xk[kt][:, hh * NW:(hh + 1) * NW],
                                start=(kt == 0), stop=(kt == KJ - 1))
                    if mt < NJ:
                        j = mt
                        xh = cnv.tile([128, L], f16, name="xh", tag="xh")
                        nc.scalar.activation(xh[:], ps[:], AF.Copy)
                        # causal depthwise conv, kernel KC, left pad KC-1
                        acc = None
                        for k in range(KC):
                            sh = KC - 1 - k
                            if acc is None:
                                p = cnv.tile([128, L], f16, name="cacc", tag="cacc")
                            else:
                                p = cnv.tile([128, L], f16, name="cp", tag="cp")
                            if sh > 0:
                                nc.vector.memset(p[:, 0:sh], 0.0)
                            nc.vector.tensor_scalar(
                                out=p[:, sh:L], in0=xh[:, 0:L - sh],
                                scalar1=convw_ap(j, k), scalar2=None, op0=OP.mult)
                            if acc is None:
                                acc = p
                            else:
                                nc.vector.tensor_add(acc[:], acc[:], p[:])
                        if use_silu:
                            nc.scalar.activation(xc[j][:], acc[:], AF.Silu,
                                                 bias=convb_ap(j))
                        else:
                            v = cnv.tile([128, L], f16, name="cv", tag="cv")
                            nc.scalar.activation(v[:], acc[:], AF.Identity,
                                                 bias=convb_ap(j))
                            sg = cnv.tile([128, L], f16, name="csg", tag="csg")
                            nc.scalar.activation(sg[:], v[:], AF.Sigmoid)
                            nc.vector.tensor_mul(xc[j][:], v[:], sg[:])
                        # xproj partial accumulation over j
                        for hh in range(NH):
                            nc.tensor.matmul(
                                ps_proj[:, hh * NW:(hh + 1) * NW],
                                xpw[j][:, :],
                                xc[j][:, hh * NW:(hh + 1) * NW],
                                start=(j == 0), stop=(j == NJ - 1))
                    else:
                        j = mt - NJ
                        if use_silu:
                            nc.scalar.activation(zs[j][:], ps[:], AF.Silu)
                        else:
                            sg = cnv.tile([128, L], f16, name="zsg", tag="zsg")
                            nc.scalar.activation(sg[:], ps[:], AF.Sigmoid)
                            nc.vector.tensor_mul(zs[j][:], sg[:], ps[:])

            # ---------------- stage B: allreduce + delta + B/C rows --------
            proj_sb = pers.tile([PROJ, L], f32, name="proj_sb", tag="proj_sb")
            nc.scalar.activation(proj_sb[:], ps_proj[:], AF.Copy)
            bounce_in = dramp.tile([PROJ, L], f32)
            bounce_out = dramp.tile([PROJ, L], f32)
            nc.sync.dma_start(bounce_in[:], proj_sb[:])
            if no_collective:
                nc.sync.dma_start(bounce_out[:], bounce_in[:])
            else:
                groups = [[2 * g, 2 * g + 1] for g in range(n_cores // 2)]
                nc.gpsimd.collective_compute(
                    "AllReduce", mybir.AluOpType.add, replica_groups=groups,
                    ins=[bounce_in.opt()], outs=[bounce_out.opt()])
            nc.sync.dma_start(projf[:], bounce_out[:])
            nc.scalar.activation(projh[:], projf[:], AF.Copy)
            nc.sync.dma_start(rows_dram[:], projh[R:PROJ, :])

            with tc.tile_pool(name="dtw", bufs=1) as dtwp, \
                 tc.tile_pool(name="psD", bufs=2, space="PSUM") as psD, \
                 tc.tile_pool(name="sptmp", bufs=2) as sptmp:
                dtw = dtwp.tile([R, DL], f16)
                nc.sync.dma_start(dtw[:], dtwT_d[:])
                for j in range(NJ):
                    ps = psD.tile([128, L], f32, name="psD", tag="psD")
                    for hh in range(NH):
                        nc.tensor.matmul(ps[:, hh * NW:(hh + 1) * NW],
                                         dtw[:, j * 128:(j + 1) * 128],
                                         projh[0:R, hh * NW:(hh + 1) * NW],
                                         start=True, stop=True)
                    # softplus(x + dtb) = Ln(Exp(x + dtb) + 1)
                    e = sptmp.tile([128, L], f32, name="spe", tag="spe")
                    nc.scalar.activation(e[:], ps[:], AF.Exp, bias=dtb_ap(j))
                    nc.scalar.activation(delta[j][:], e[:], AF.Ln, bias=one_t[:])
                    nc.vector.tensor_mul(du[j][:], delta[j][:], xc[j][:])

            # ---------------- stage C: scan block --------------------------
            # j outer / s inner; per-j y accumulates in PSUM via PE
            # identity-matmuls over the hC slices (+ diag(D) @ xc term).
            GW = NB * W1
            with tc.tile_pool(name="bc", bufs=1) as bcp, \
                 tc.tile_pool(name="sc", bufs=2) as scp, \
                 tc.tile_pool(name="dd", bufs=2) as ddp, \
                 tc.tile_pool(name="psY", bufs=2, space="PSUM") as psY:
                Ball = bcp.tile([128, NN * L], f16, name="Ball", tag="Ball")
                Call = bcp.tile([128, NN * L], f16, name="Call", tag="Call")
                for n in range(NN):
                    nc.sync.dma_start(Ball[:, n * L:(n + 1) * L],
                                      rows_dram[n, :].partition_broadcast(128))
                    nc.sync.dma_start(Call[:, n * L:(n + 1) * L],
                                      rows_dram[NN + n, :].partition_broadcast(128))
                for j in range(NJ):
                    dd = ddp.tile([128, 128], f16, name="dd", tag="dd")
                    nc.sync.dma_start(dd[:], ddiag_d[j])
                    ps_y = psY.tile([128, L], f32, name="ps_y", tag="ps_y")
                    for hh in range(NH):
                        nc.tensor.matmul(ps_y[:, hh * NW:(hh + 1) * NW], dd[:],
                                         xc[j][:, hh * NW:(hh + 1) * NW],
                                         start=True, stop=False)
                    for s in range(NGRP):
                        ns = [s * NB + i for i in range(NB)]
                        Bv = Ball[:, s * NB * L:(s + 1) * NB * L].rearrange(
                            "p (n l) -> p n l", n=NB)
                        Cv = Call[:, s * NB * L:(s + 1) * NB * L].rearrange(
                            "p (n l) -> p n l", n=NB)
                        dA = scp.tile([128, GW], f16, name="dA", tag="dA")
                        dbu = scp.tile([128, GW], f16, name="dbu", tag="dbu")
                        dAv = dA[:].rearrange("p (n w) -> p n w", n=NB)
                        dbv = dbu[:].rearrange("p (n w) -> p n w", n=NB)
                        nc.vector.memset(dAv[:, :, L:W1], 0.0)
                        nc.vector.memset(dbv[:, :, L:W1], 0.0)
                        for i, n in enumerate(ns):
                            nc.scalar.activation(dA[:, i * W1:i * W1 + L],
                                                 delta[j][:], AF.Exp,
                                                 scale=A_ap(j, n))
                        nc.vector.tensor_mul(
                            dbv[:, :, 0:L],
                            du[j][:, None, :].broadcast_to([128, NB, L]), Bv)
                        nc.vector.tensor_tensor_scan(
                            dbu[:], dA[:], dbu[:], 0.0, OP.mult, OP.add)
                        # h is now in dbu; multiply by C in place
                        if s == NGRP - 1:
                            nc.gpsimd.tensor_mul(dbv[:, :, 0:L], dbv[:, :, 0:L], Cv)
                        else:
                            nc.vector.tensor_mul(dbv[:, :, 0:L], dbv[:, :, 0:L], Cv)
                        # accumulate the NB states into ps_y on the PE
                        for i in range(NB):
                            last = (s == NGRP - 1 and i == NB - 1)
                            for hh in range(NH):
                                nc.tensor.matmul(
                                    ps_y[:, hh * NW:(hh + 1) * NW], ident[:],
                                    dbv[:, i, hh * NW:(hh + 1) * NW],
                                    start=False, stop=last)
                    # y_full = (y_scan + xc*D) * silu(z), overwriting zs[j]
                    nc.vector.tensor_mul(zs[j][:], zs[j][:], ps_y[:])

            # ---------------- stage D: out_proj ----------------------------
            with tc.tile_pool(name="ow", bufs=1) as owp, \
                 tc.tile_pool(name="psO", bufs=2, space="PSUM") as psO, \
                 tc.tile_pool(name="osb", bufs=2) as osbp:
                ow = []
                for kt in range(NJ):
                    t = owp.tile([128, DM], f16, name=f"ow{kt}", tag=f"ow{kt}")
                    nc.sync.dma_start(t[:], outwT_d[kt * 128:(kt + 1) * 128, :])
                    ow.append(t)
                for mt in range(KJ):
                    ps = psO.tile([128, L], f32, name="psO", tag="psO")
                    for kt in range(NJ):
                        for hh in range(NH):
                            nc.tensor.matmul(
                                ps[:, hh * NW:(hh + 1) * NW],
                                ow[kt][:, mt * 128:(mt + 1) * 128],
                                zs[kt][:, hh * NW:(hh + 1) * NW],
                                start=(kt == 0), stop=(kt == NJ - 1))
                    osb = osbp.tile([128, L], f32, name="osb", tag="osb")
                    nc.scalar.activation(osb[:], ps[:], AF.Copy)
                    nc.sync.dma_start(out_d[mt * 128:(mt + 1) * 128, :], osb[:])

    nc.compile()
    return nc


# ---------------------------------------------------------------- host ------

def shard_inputs(inputs, DM, DI, L, NN, R, KC):
    """Build the 8 per-core input maps from the full input dict."""
    import ml_dtypes
    f16 = np.dtype(F16)
    DL = DI // 2
    NJ = DL // 128
    PPRM = NN + KC + 3
    x = np.asarray(inputs["x"], np.float32)

    in_maps = []
    for c in range(N_CORES):
        b, d, h = c // 4, (c // 2) % 2, c % 2
        p = "f" if d == 0 else "b"
        g = lambda k: np.asarray(inputs[f"{p}_{k}"], np.float32)
        xs = x[b] if d == 0 else x[b, ::-1]
        lo, hi = h * DL, (h + 1) * DL

        in_w = g("in_w")
        inwT = np.concatenate([in_w[lo:hi], in_w[DI + lo:DI + hi]], 0).T
        A = -np.exp(g("A_log")[lo:hi])
        prm = np.zeros((NJ, 128, PPRM), np.float32)
        ddiag = np.zeros((NJ, 128, 128), np.float32)
        for j in range(NJ):
            r = slice(j * 128, (j + 1) * 128)
            prm[j, :, 0:NN] = A[r]
            prm[j, :, NN:NN + KC] = g("conv_w")[lo:hi][r]
            prm[j, :, NN + KC] = g("conv_b")[lo:hi][r]
            prm[j, :, NN + KC + 1] = g("dt_b")[lo:hi][r]
            prm[j, :, NN + KC + 2] = g("D")[lo:hi][r]
            np.fill_diagonal(ddiag[j], g("D")[lo:hi][r])

        in_maps.append({
            "ident": np.eye(128, dtype=np.float32).astype(f16),
            "ddiag": ddiag.astype(f16),
            "xT": np.ascontiguousarray(xs.T).astype(f16),
            "inwT": np.ascontiguousarray(inwT).astype(f16),
            "xprojT": np.ascontiguousarray(g("xproj_w")[:, lo:hi].T).astype(f16),
            "dtwT": np.ascontiguousarray(g("dt_w")[lo:hi].T).astype(f16),
            "outwT": np.ascontiguousarray(g("out_w")[:, lo:hi].T).astype(f16),
            "prm": prm,
        })
    return in_maps


def unshard_outputs(results, B, L, DM):
    y = np.zeros((B, L, DM), np.float32)
    for c in range(N_CORES):
        b, d = c // 4, (c // 2) % 2
        part = results[c]["out"].T  # [L, DM]
        y[b] += part if d == 0 else part[::-1]
    return y


# --------------------------------------------------------------- kernel -----

_CACHE = {}


def kernel(**inputs):
    from concourse.bass_utils import run_bass_kernel_spmd
    cfg = FULL
    key = "full"
    if key not in _CACHE:
        _CACHE[key] = build_program(**cfg)
    nc = _CACHE[key]
    in_maps = shard_inputs(inputs, **cfg)
    res = run_bass_kernel_spmd(nc, in_maps, list(range(N_CORES)))
    out = unshard_outputs(res.results, 2, cfg["L"], cfg["DM"])
    return out.astype(np.asarray(inputs["x"]).dtype)


# revision 45
# speedup vs baseline: 3474.9078x; 3474.9078x over previous
"""BiMamba block (fwd + bwd Mamba on [2, 1024, 1024]) for 8 Trainium2 NeuronCores.

Sharding: core = (batch b, direction d, channel-half h)  ->  c = b*4 + d*2 + h.
Each core runs one full Mamba direction on one batch element with half the
d_inner channels (1024 of 2048).  The only cross-core exchange is a 2-core
AllReduce of the x-projection partials ([96, L] fp32) between the two
channel-halves of the same (batch, direction).  The depthwise conv, SSM scan,
and gating are channel-local.  Final out-proj partials ([D_MODEL, L] fp32 per
core) are summed on the host.

Layout on chip: channels on partitions (8 j-tiles of 128), sequence L on the
free dim.  The selective scan runs as DVE tensor_tensor_scan ops over glued
[128, NB*(L+1)] tiles (n-states side by side, zero seam column resets the
recurrence between states).
"""

import numpy as np

# ---------------------------------------------------------------- config ----

FULL = dict(DM=1024, DI=2048, L=1024, NN=16, R=64, KC=4)

N_CORES = 8
NB = 4          # n-states per glued scan group
F16 = "float16"  # on-chip low-precision dtype


# ------------------------------------------------------------- program ------

def build_program(DM, DI, L, NN, R, KC, use_silu=True, n_cores=N_CORES,
                  no_collective=False):
    """Emit the per-core Tile program (SPMD, identical on all cores)."""
    import concourse.bass as bass
    import concourse.mybir as mybir
    import concourse.tile as tile
    from concourse import bacc

    dt = mybir.dt
    f32 = dt.float32
    f16 = getattr(dt, F16)
    AF = mybir.ActivationFunctionType
    OP = mybir.AluOpType

    DL = DI // 2          # local d_inner channels
    NJ = DL // 128        # channel tiles
    KJ = DM // 128        # d_model tiles
    PROJ = R + 2 * NN     # 96
    W1 = L + 1            # glued per-state width (incl. seam)
    NGRP = NN // NB
    NH = max(L // 512, 1) # 512-wide matmul halves
    NW = min(L, 512)

    nc = bacc.Bacc("TRN2", target_bir_lowering=False, debug=False,
                   num_devices=n_cores)

    dram = lambda name, shape, d, kind: nc.dram_tensor(name, shape, d, kind=kind).ap()
    xT_d = dram("xT", [DM, L], f16, "ExternalInput")
    inwT_d = dram("inwT", [2 * DL // 128, 128, DM], f16, "ExternalInput")
    xprojT_d = dram("xprojT", [DL, PROJ], f16, "ExternalInput")
    dtwT_d = dram("dtwT", [R, DL], f16, "ExternalInput")
    outwT_d = dram("outwT", [KJ, 128, DL], f16, "ExternalInput")
    ddiag_d = dram("ddiag", [NJ, 128, 128], f16, "ExternalInput")
    ident_d = dram("ident", [128, 128], f16, "ExternalInput")
    # packed per-j params: cols 0:NN A | NN:NN+KC convw | +convb | +dtb | +Dp
    PPRM = NN + KC + 3
    prm_d = dram("prm", [NJ, 128, PPRM], f32, "ExternalInput")
    out_d = dram("out", [DM, L], f32, "ExternalOutput")

    with tile.TileContext(nc) as tc:
        import contextlib
        ctx = contextlib.ExitStack()
        with ctx:
            # ---------------- persistent pools ----------------
            pers = ctx.enter_context(tc.tile_pool(name="pers", bufs=1))
            dramp = ctx.enter_context(tc.tile_pool(name="dram", bufs=1, space="DRAM"))

            ctxX = contextlib.ExitStack()
            xcp = ctxX.enter_context(tc.tile_pool(name="xcp", bufs=1))
            xc = [xcp.tile([128, L], f16, name=f"xc{j}", tag=f"xc{j}") for j in range(NJ)]
            xc_dram = dramp.tile([NJ, 128, L], f16)
            zs = [pers.tile([128, L], f16, name=f"zs{j}", tag=f"zs{j}") for j in range(NJ)]
            delta = [pers.tile([128, L], f16, name=f"dl{j}", tag=f"dl{j}") for j in range(NJ)]
            du = [pers.tile([128, L], f16, name=f"du{j}", tag=f"du{j}") for j in range(NJ)]
            prm = [pers.tile([128, PPRM], f32, name=f"pr{j}", tag=f"pr{j}") for j in range(NJ)]
            ident = pers.tile([128, 128], f16, name="ident", tag="ident")
            nc.sync.dma_start(ident[:], ident_d[:])
            one_t = pers.tile([128, 1], f32, name="one", tag="one")
            nc.vector.memset(one_t[:], 1.0)
            for j in range(NJ):
                nc.sync.dma_start(prm[j][:], prm_d[j])
            A_ap = lambda j, n: prm[j][:, n:n + 1]
            convw_ap = lambda j, k: prm[j][:, NN + k:NN + k + 1]
            convb_ap = lambda j: prm[j][:, NN + KC:NN + KC + 1]
            dtb_ap = lambda j: prm[j][:, NN + KC + 1:NN + KC + 2]
            Dp_ap = lambda j: prm[j][:, NN + KC + 2:NN + KC + 3]

            projh = pers.tile([R, L], f16, name="projh", tag="projh")

            # ---------------- stage A: in_proj + conv + silu ----------------
            ctxP = contextlib.ExitStack()
            psP = ctxP.enter_context(tc.tile_pool(name="psP", bufs=1, space="PSUM"))
            ps_proj = psP.tile([PROJ, L], f32)

            with tc.tile_pool(name="xk", bufs=1) as xkp, \
                 tc.tile_pool(name="wk", bufs=1) as wkp, \
                 tc.tile_pool(name="xpw", bufs=1) as xpwp, \
                 tc.tile_pool(name="psA", bufs=3, space="PSUM") as psA, \
                 tc.tile_pool(name="cnv", bufs=2) as cnv:
                xk = []
                win_pre = []
                for mt in range(2):
                    w = wkp.tile([128, DM], f16, name="win", tag="win", bufs=3)
                    nc.sync.dma_start(w[:], inwT_d[mt])
                    win_pre.append(w)
                for kt in range(KJ):
                    t = xkp.tile([128, L], f16, name=f"xk{kt}", tag=f"xk{kt}")
                    nc.sync.dma_start(t[:], xT_d[kt * 128:(kt + 1) * 128, :])
                    xk.append(t)
                xpw = []
                for j in range(NJ):
                    t = xpwp.tile([128, PROJ], f16, name=f"xpw{j}", tag=f"xpw{j}")
                    nc.sync.dma_start(t[:], xprojT_d[j * 128:(j + 1) * 128, :])
                    xpw.append(t)

                dtw = xpwp.tile([R, DL], f16, name="dtw", tag="dtw")
                nc.sync.dma_start(dtw[:], dtwT_d[:])

                def emit_mtile(mt):
                    if mt < 2:
                        win = win_pre[mt]
                    else:
                        win = wkp.tile([128, DM], f16, name="win", tag="win",
                                       bufs=3)
                        nc.sync.dma_start(win[:], inwT_d[mt])
                    ps = psA.tile([128, L], f32, name="psA", tag="psA")
                    for kt in range(KJ):
                        for hh in range(NH):
                            nc.tensor.matmul(
                                ps[:, hh * NW:(hh + 1) * NW],
                                win[:, kt * 128:(kt + 1) * 128],
                                xk[kt][:, hh * NW:(hh + 1) * NW],
                                start=(kt == 0), stop=(kt == KJ - 1))
                    if mt < NJ:
                        j = mt
                        xh = cnv.tile([128, L], f16, name="xh", tag="xh")
                        nc.scalar.activation(xh[:], ps[:], AF.Copy)
                        # causal depthwise conv, kernel KC, left pad KC-1
                        acc = None
                        for k in range(KC):
                            sh = KC - 1 - k
                            if acc is None:
                                p = cnv.tile([128, L], f16, name="cacc", tag="cacc")
                            else:
                                p = cnv.tile([128, L], f16, name="cp", tag="cp")
                            if sh > 0:
                                nc.vector.memset(p[:, 0:sh], 0.0)
                            nc.vector.tensor_scalar(
                                out=p[:, sh:L], in0=xh[:, 0:L - sh],
                                scalar1=convw_ap(j, k), scalar2=None, op0=OP.mult)
                            if acc is None:
                                acc = p
                            else:
                                nc.vector.tensor_add(acc[:], acc[:], p[:])
                        if use_silu:
                            nc.scalar.activation(xc[j][:], acc[:], AF.Silu,
                                                 bias=convb_ap(j))
                        else:
                            v = cnv.tile([128, L], f16, name="cv", tag="cv")
                            nc.scalar.activation(v[:], acc[:], AF.Identity,
                                                 bias=convb_ap(j))
                            sg = cnv.tile([128, L], f16, name="csg", tag="csg")
                            nc.scalar.activation(sg[:], v[:], AF.Sigmoid)
                            nc.vector.tensor_mul(xc[j][:], v[:], sg[:])
                        # xproj partial accumulation over j
                        for hh in range(NH):
                            nc.tensor.matmul(
                                ps_proj[:, hh * NW:(hh + 1) * NW],
                                xpw[j][:, :],
                                xc[j][:, hh * NW:(hh + 1) * NW],
                                start=(j == 0), stop=(j == NJ - 1))
                    else:
                        # z tile: raw silu input parked in zs[j]; silu applied
                        # in place later (keeps the ACT table sequence clean)
                        j = mt - NJ
                        nc.vector.tensor_copy(zs[j][:], ps[:])

                for mt in range(NJ + 2):
                    emit_mtile(mt)

                # -------- stage B: allreduce + delta (before remaining z) ---
                proj_sb = pers.tile([PROJ, L], f16, name="proj_sb", tag="proj_sb")
                nc.scalar.activation(proj_sb[:], ps_proj[:], AF.Copy)
                bounce_in = dramp.tile([PROJ, L], f16)
                bounce_out = dramp.tile([PROJ, L], f16)
                nc.sync.dma_start(bounce_in[:], proj_sb[:])
                if no_collective:
                    nc.sync.dma_start(bounce_out[:], bounce_in[:])
                else:
                    groups = [[2 * g, 2 * g + 1] for g in range(n_cores // 2)]
                    nc.gpsimd.collective_compute(
                        "AllReduce", mybir.AluOpType.add, replica_groups=groups,
                        ins=[bounce_in.opt()], outs=[bounce_out.opt()])
                nc.sync.dma_start(projh[:], bounce_out[0:R, :])
                rows_dram = bounce_out

                with tc.tile_pool(name="sptmp", bufs=2) as sptmp:
                    for j in range(NJ):
                        ps = psA.tile([128, L], f32, name="psD", tag="psA")
                        for hh in range(NH):
                            nc.tensor.matmul(ps[:, hh * NW:(hh + 1) * NW],
                                             dtw[:, j * 128:(j + 1) * 128],
                                             projh[0:R, hh * NW:(hh + 1) * NW],
                                             start=True, stop=True)
                        # softplus(x + dtb) = Ln(Exp(x + dtb) + 1)
                        e = sptmp.tile([128, L], f32, name="spe", tag="spe")
                        nc.scalar.activation(e[:], ps[:], AF.Exp, bias=dtb_ap(j))
                        nc.scalar.activation(delta[j][:], e[:], AF.Ln,
                                             bias=one_t[:])
                        nc.gpsimd.tensor_mul(du[j][:], delta[j][:], xc[j][:])
                        nc.sync.dma_start(xc_dram[j], xc[j][:])

                for mt in range(NJ + 2, 2 * NJ):
                    emit_mtile(mt)

            ctxP.close()
            ctxX.close()

            def emit_zsilu():
                for j in range(NJ):
                    if use_silu:
                        nc.scalar.activation(zs[j][:], zs[j][:], AF.Silu)
                    else:
                        sg2 = scp.tile([128, L], f16, name="zsg2", tag="zsg2",
                                       bufs=2)
                        nc.scalar.activation(sg2[:], zs[j][:], AF.Sigmoid)
                        nc.vector.tensor_mul(zs[j][:], sg2[:], zs[j][:])

            # ---------------- stage C: scan block --------------------------
            # j outer / s inner; per-j y accumulates in PSUM via PE
            # identity-matmuls over the hC slices (+ diag(D) @ xc term).
            GW = NB * W1
            with tc.tile_pool(name="bc", bufs=1) as bcp, \
                 tc.tile_pool(name="sc", bufs=4) as scp, \
                 tc.tile_pool(name="dd", bufs=2) as ddp, \
                 tc.tile_pool(name="psY", bufs=2, space="PSUM") as psY, \
                 tc.tile_pool(name="owm", bufs=1) as owmp, \
                 tc.tile_pool(name="psO", bufs=2, space="PSUM") as psO, \
                 tc.tile_pool(name="osb", bufs=1) as osbp:
                Ball = bcp.tile([128, NN * L], f16, name="Ball", tag="Ball")
                Call = bcp.tile([128, NN * L], f16, name="Call", tag="Call")
                for s0 in range(NGRP):
                    for n in range(s0 * NB, (s0 + 1) * NB):
                        nc.sync.dma_start(Ball[:, n * L:(n + 1) * L],
                                          rows_dram[R + n, :].partition_broadcast(128))
                    for n in range(s0 * NB, (s0 + 1) * NB):
                        nc.gpsimd.dma_start(Call[:, n * L:(n + 1) * L],
                                            rows_dram[R + NN + n, :].partition_broadcast(128))
                # out_proj weights + split bookkeeping (stage D overlaps C)
                KT1 = max(NJ - 3, 0)  # pass-1 contraction depth (kt 0..KT1-1)
                owm = []
                for mt in range(KJ):
                    t = owmp.tile([128, DL], f16, name=f"owm{mt}", tag=f"owm{mt}")
                    nc.sync.dma_start(t[:], outwT_d[mt])
                    owm.append(t)
                op1 = [None] * KJ
                op1_dram = dramp.tile([KJ, 128, L], f16)
                psy_tiles = [None] * NJ

                def emit_yfull(j):
                    # y_full = (y_scan + xc*D) * silu(z), overwriting zs[j]
                    nc.vector.tensor_mul(zs[j][:], zs[j][:], psy_tiles[j][:])

                def emit_pass1(mt):
                    # partial out_proj over kt < KT1, parked in SBUF as f16
                    ps = psO.tile([128, L], f32, name="psO", tag="psO")
                    for kt in range(KT1):
                        for hh in range(NH):
                            nc.tensor.matmul(
                                ps[:, hh * NW:(hh + 1) * NW],
                                owm[mt][:, kt * 128:(kt + 1) * 128],
                                zs[kt][:, hh * NW:(hh + 1) * NW],
                                start=(kt == 0), stop=(kt == KT1 - 1))
                    t = osbp.tile([128, L], f16, name="op1t", tag="op1t", bufs=1)
                    nc.scalar.activation(t[:], ps[:], AF.Copy)
                    nc.sync.dma_start(op1_dram[mt], t[:])
                    op1[mt] = True

                for j in range(NJ):
                    dd = ddp.tile([128, 128], f16, name="dd", tag="dd")
                    nc.sync.dma_start(dd[:], ddiag_d[j])
                    xcr = ddp.tile([128, L], f16, name="xcr", tag="xcr")
                    nc.sync.dma_start(xcr[:], xc_dram[j])
                    ps_y = psY.tile([128, L], f32, name="ps_y", tag="ps_y")
                    psy_tiles[j] = ps_y
                    for hh in range(NH):
                        nc.tensor.matmul(ps_y[:, hh * NW:(hh + 1) * NW], dd[:],
                                         xcr[:, hh * NW:(hh + 1) * NW],
                                         start=True, stop=False)
                    for s in range(NGRP):
                        ns = [s * NB + i for i in range(NB)]
                        Bv = Ball[:, s * NB * L:(s + 1) * NB * L].rearrange(
                            "p (n l) -> p n l", n=NB)
                        Cv = Call[:, s * NB * L:(s + 1) * NB * L].rearrange(
                            "p (n l) -> p n l", n=NB)
                        dA = scp.tile([128, GW], f16, name="dA", tag="dA")
                        dbu = scp.tile([128, GW], f16, name="dbu", tag="dbu")
                        dAv = dA[:].rearrange("p (n w) -> p n w", n=NB)
                        dbv = dbu[:].rearrange("p (n w) -> p n w", n=NB)
                        nc.vector.memset(dAv[:, :, L:W1], 0.0)
                        nc.vector.memset(dbv[:, :, L:W1], 0.0)
                        for i, n in enumerate(ns):
                            nc.scalar.activation(dA[:, i * W1:i * W1 + L],
                                                 delta[j][:], AF.Exp,
                                                 scale=A_ap(j, n))
                        nc.vector.tensor_mul(
                            dbv[:, :, 0:L],
                            du[j][:, None, :].broadcast_to([128, NB, L]), Bv)
                        nc.vector.tensor_tensor_scan(
                            dbu[:], dA[:], dbu[:], 0.0, OP.mult, OP.add)
                        # h is now in dbu; multiply by C in place
                        if s < NGRP - 1:
                            nc.gpsimd.tensor_mul(dbv[:, :, 0:L], dbv[:, :, 0:L], Cv)
                        else:
                            nc.vector.tensor_mul(dbv[:, :, 0:L], dbv[:, :, 0:L], Cv)
                        # accumulate the NB states into ps_y on the PE
                        for i in range(NB):
                            last = (s == NGRP - 1 and i == NB - 1)
                            for hh in range(NH):
                                nc.tensor.matmul(
                                    ps_y[:, hh * NW:(hh + 1) * NW], ident[:],
                                    dbv[:, i, hh * NW:(hh + 1) * NW],
                                    start=False, stop=last)
                        if j == 0 and s == 0:
                            emit_zsilu()
                    # defer yfull by one j so DVE never stalls on the PE here
                    if j >= 1:
                        emit_yfull(j - 1)
                    if NJ - 2 <= j <= NJ - 1 and KT1 >= 1:
                        half = KJ // 2
                        for mt in range(half * (j - (NJ - 2)),
                                        half * (j - (NJ - 2)) + half):
                            emit_pass1(mt)
                emit_yfull(NJ - 1)

                # ------------ stage D tail: remaining kt + combine ----------
                for mt in range(KJ):
                    ps = psO.tile([128, L], f32, name="psO", tag="psO")
                    for kt in range(KT1, NJ):
                        for hh in range(NH):
                            nc.tensor.matmul(
                                ps[:, hh * NW:(hh + 1) * NW],
                                owm[mt][:, kt * 128:(kt + 1) * 128],
                                zs[kt][:, hh * NW:(hh + 1) * NW],
                                start=(kt == KT1), stop=(kt == NJ - 1))
                    osb = osbp.tile([128, L], f32, name="osb", tag="osb", bufs=1)
                    if op1[mt] is None:
                        nc.scalar.activation(osb[:], ps[:], AF.Copy)
                    else:
                        t2 = osbp.tile([128, L], f16, name="op1r", tag="op1t", bufs=1)
                        nc.sync.dma_start(t2[:], op1_dram[mt])
                        nc.vector.tensor_add(osb[:], ps[:], t2[:])
                    nc.sync.dma_start(out_d[mt * 128:(mt + 1) * 128, :], osb[:])

    nc.compile()
    return nc


# ---------------------------------------------------------------- host ------

def shard_inputs(inputs, DM, DI, L, NN, R, KC):
    """Build the 8 per-core input maps from the full input dict."""
    f16 = np.dtype(F16)
    DL = DI // 2
    NJ = DL // 128
    PPRM = NN + KC + 3
    x = np.asarray(inputs["x"], np.float32)

    in_maps = []
    for c in range(N_CORES):
        b, d, h = c // 4, (c // 2) % 2, c % 2
        p = "f" if d == 0 else "b"
        g = lambda k: np.asarray(inputs[f"{p}_{k}"], np.float32)
        xs = x[b] if d == 0 else x[b, ::-1]
        lo, hi = h * DL, (h + 1) * DL

        in_w = g("in_w")
        inwT = np.concatenate([in_w[lo:hi], in_w[DI + lo:DI + hi]], 0).T
        NMT, KJh = (2 * DL) // 128, DM // 128
        inw_pack = (inwT.reshape(KJh, 128, NMT, 128)
                    .transpose(2, 1, 0, 3).reshape(NMT, 128, DM))
        A = -np.exp(g("A_log")[lo:hi])
        prm = np.zeros((NJ, 128, PPRM), np.float32)
        ddiag = np.zeros((NJ, 128, 128), np.float32)
        for j in range(NJ):
            r = slice(j * 128, (j + 1) * 128)
            prm[j, :, 0:NN] = A[r]
            prm[j, :, NN:NN + KC] = g("conv_w")[lo:hi][r]
            prm[j, :, NN + KC] = g("conv_b")[lo:hi][r]
            prm[j, :, NN + KC + 1] = g("dt_b")[lo:hi][r]
            prm[j, :, NN + KC + 2] = g("D")[lo:hi][r]
            np.fill_diagonal(ddiag[j], g("D")[lo:hi][r])

        in_maps.append({
            "ident": np.eye(128, dtype=np.float32).astype(f16),
            "ddiag": ddiag.astype(f16),
            "xT": np.ascontiguousarray(xs.T).astype(f16),
            "inwT": np.ascontiguousarray(inw_pack).astype(f16),
            "xprojT": np.ascontiguousarray(g("xproj_w")[:, lo:hi].T).astype(f16),
            "dtwT": np.ascontiguousarray(g("dt_w")[lo:hi].T).astype(f16),
            "outwT": np.ascontiguousarray(
                g("out_w")[:, lo:hi].reshape(DM // 128, 128, DL // 128, 128)
                .transpose(0, 3, 2, 1).reshape(DM // 128, 128, DL)).astype(f16),
            "prm": prm,
        })
    return in_maps


def unshard_outputs(results, B, L, DM):
    y = np.zeros((B, L, DM), np.float32)
    for c in range(N_CORES):
        b, d = c // 4, (c // 2) % 2
        part = results[c]["out"].T  # [L, DM]
        y[b] += part if d == 0 else part[::-1]
    return y


# --------------------------------------------------------------- kernel -----

_CACHE = {}


def kernel(**inputs):
    from concourse.bass_utils import run_bass_kernel_spmd
    cfg = FULL
    key = "full"
    if key not in _CACHE:
        _CACHE[key] = build_program(**cfg)
    nc = _CACHE[key]
    in_maps = shard_inputs(inputs, **cfg)
    res = run_bass_kernel_spmd(nc, in_maps, list(range(N_CORES)))
    out = unshard_outputs(res.results, 2, cfg["L"], cfg["DM"])
    return out.astype(np.asarray(inputs["x"]).dtype)
